# revision 7
# baseline (speedup 1.0000x reference)
import os
import sys
import time
import numpy as np
from contextlib import ExitStack

for _p in ("/opt/trn_rl_repo", "/root/.axon_site/_ro/trn_rl_repo"):
    if os.path.isdir(_p) and _p not in sys.path:
        sys.path.append(_p)

import ml_dtypes

BF16 = ml_dtypes.bfloat16

D = 256
H = 4
DH = 64
N_SRC = 100000
N_DST = 50000
N_EDGES = 300000
NDEV = 8
DST_PER_DEV = N_DST // NDEV  # 6250
NBLK = (DST_PER_DEV + 127) // 128  # 49
DST_PAD = NBLK * 128  # 6272

LAST_EXEC_NS = None
_TV = bool(os.environ.get("KERNEL_TIMING"))


def _tlog(msg, t0):
    if _TV:
        print(f"[ktime] {msg} {time.time() - t0:.2f}s", flush=True)


def _prep_host(h_src, h_dst, src_idx, dst_idx, Wq, bq, Wk, bk, Wv, bv):
    """Returns concat-level input arrays (axis 0 = per-core slices) + C."""
    order = np.argsort(dst_idx, kind="stable")
    sdst = dst_idx[order]
    bounds = np.searchsorted(sdst, np.arange(0, N_DST + 1, DST_PER_DEV))

    per_dev = []
    C = 1
    for d in range(NDEV):
        lo, hi = int(bounds[d]), int(bounds[d + 1])
        local = (sdst[lo:hi] - d * DST_PER_DEV).astype(np.int64)
        blk = local // 128
        cnt = np.bincount(blk, minlength=NBLK)
        if cnt.max() > 0:
            C = max(C, int(np.ceil(cnt.max() / 128.0)))
        per_dev.append((lo, hi, local, blk, cnt))

    WKVT = np.ascontiguousarray(
        np.concatenate([Wk.T, Wv.T], axis=1).reshape(2, 128, 512).transpose(1, 0, 2)
    ).astype(BF16)
    WQT = np.ascontiguousarray(
        Wq.T.reshape(2, 128, 256).transpose(1, 0, 2)).astype(BF16)
    has_bias = bool(np.any(bk) or np.any(bv) or np.any(bq))

    h_src_bf = h_src.astype(BF16)
    h_dst_bf = h_dst.astype(BF16)

    nchunks = NBLK * C
    E_pad = nchunks * 128
    X_all = np.empty((NDEV * 128, nchunks, 2, 128), BF16)
    DL_all = np.empty((NDEV * 128, NBLK, C), np.float32)
    HD_all = np.empty((NDEV * 128, NBLK, 2, 128), BF16)
    for d in range(NDEV):
        lo, hi, local, blk, cnt = per_dev[d]
        starts = np.concatenate([[0], np.cumsum(cnt)[:-1]])
        pos = np.arange(hi - lo) - starts[blk]
        slot = blk * (C * 128) + pos

        eids = order[lo:hi]
        Xf = np.zeros((E_pad, D), BF16)
        Xf[slot] = h_src_bf[src_idx[eids]]
        # [128(emb-half), nchunks, 2, 128(slot)]: partition = embedding so the
        # KV matmul contracts over it; each partition DMAs long contiguous rows
        X_all[d * 128:(d + 1) * 128] = \
            Xf.reshape(nchunks, 128, 2, 128).transpose(3, 0, 2, 1)

        dl = np.full((NBLK, C, 128), 128.0, np.float32)
        dl.reshape(E_pad)[slot] = (local % 128).astype(np.float32)
        DL_all[d * 128:(d + 1) * 128] = dl.transpose(2, 0, 1)

        hd = np.zeros((DST_PAD, D), BF16)
        hd[:DST_PER_DEV] = h_dst_bf[d * DST_PER_DEV:(d + 1) * DST_PER_DEV]
        HD_all[d * 128:(d + 1) * 128] = \
            hd.reshape(NBLK, 128, 2, 128).transpose(3, 0, 2, 1)

    arrs = {
        "X": X_all,
        "DLOC": DL_all,
        "HD": HD_all,
        "WKV": np.tile(WKVT, (NDEV, 1, 1)),
        "WQ": np.tile(WQT, (NDEV, 1, 1)),
        "BKV": np.tile(np.concatenate([bk, bv]).astype(BF16).reshape(1, 512),
                       (NDEV, 1)),
        "BQ": np.tile(bq.astype(BF16).reshape(1, 256), (NDEV, 1)),
        "IOTA": np.tile(np.arange(128, dtype=np.float32), (NDEV * 128, 1)),
        "IDENT": np.tile(np.eye(128, dtype=np.float32), (NDEV, 1)),
    }
    return arrs, C, has_bias


def _build(C, has_bias):
    from concourse import bacc, bass, mybir, tile

    F32 = mybir.dt.float32
    BF = mybir.dt.bfloat16
    nchunks = NBLK * C
    nc = bacc.Bacc(trn_type="TRN2")
    X_d = nc.dram_tensor("X", [128, nchunks, 2, 128], BF, kind="ExternalInput")
    DL_d = nc.dram_tensor("DLOC", [128, NBLK, C], F32, kind="ExternalInput")
    HD_d = nc.dram_tensor("HD", [128, NBLK, 2, 128], BF, kind="ExternalInput")
    WKV_d = nc.dram_tensor("WKV", [128, 2, 512], BF, kind="ExternalInput")
    WQ_d = nc.dram_tensor("WQ", [128, 2, 256], BF, kind="ExternalInput")
    BKV_d = nc.dram_tensor("BKV", [1, 512], BF, kind="ExternalInput")
    BQ_d = nc.dram_tensor("BQ", [1, 256], BF, kind="ExternalInput")
    IOTA_d = nc.dram_tensor("IOTA", [128, 128], F32, kind="ExternalInput")
    IDENT_d = nc.dram_tensor("IDENT", [128, 128], F32, kind="ExternalInput")
    out_d = nc.dram_tensor("out", [NBLK, 128, 256], BF, kind="ExternalOutput")

    Copy = mybir.ActivationFunctionType.Copy
    Exp = mybir.ActivationFunctionType.Exp
    mult = mybir.AluOpType.mult
    addop = mybir.AluOpType.add
    maxop = mybir.AluOpType.max
    iseq = mybir.AluOpType.is_equal

    with ExitStack() as ctx:
        tc = ctx.enter_context(tile.TileContext(nc))
        cpool = ctx.enter_context(tc.tile_pool(name="const", bufs=1))
        bpool = ctx.enter_context(tc.tile_pool(name="blk", bufs=2))
        kpool = ctx.enter_context(tc.tile_pool(name="chunk", bufs=3))
        qpp = ctx.enter_context(tc.tile_pool(name="qps", bufs=1, space="PSUM"))
        upp = ctx.enter_context(tc.tile_pool(name="ups", bufs=2, space="PSUM"))
        kpp = ctx.enter_context(tc.tile_pool(name="kvp", bufs=2, space="PSUM"))
        gpp = ctx.enter_context(tc.tile_pool(name="qgp", bufs=2, space="PSUM"))
        app = ctx.enter_context(tc.tile_pool(name="a1p", bufs=1, space="PSUM"))

        wkv_sb = cpool.tile([128, 2, 512], BF)
        nc.sync.dma_start(out=wkv_sb, in_=WKV_d[:, :, :])
        wq_sb = cpool.tile([128, 2, 256], BF)
        nc.sync.dma_start(out=wq_sb, in_=WQ_d[:, :, :])
        iota_sb = cpool.tile([128, 128], F32)
        nc.sync.dma_start(out=iota_sb, in_=IOTA_d[:, :])
        ident_sb = cpool.tile([128, 128], F32)
        nc.sync.dma_start(out=ident_sb, in_=IDENT_d[:, :])
        dloc_sb = cpool.tile([128, NBLK, C], F32)
        nc.sync.dma_start(out=dloc_sb, in_=DL_d[:, :, :])
        if has_bias:
            ones_sb = cpool.tile([1, 128], BF)
            nc.vector.memset(ones_sb, 1.0)
            bkv_sb = cpool.tile([1, 512], BF)
            nc.sync.dma_start(out=bkv_sb, in_=BKV_d[:, :])
            bq_sb = cpool.tile([1, 256], BF)
            nc.sync.dma_start(out=bq_sb, in_=BQ_d[:, :])

        for b in range(NBLK):
            hd_sb = bpool.tile([128, 2, 128], BF)
            nc.sync.dma_start(out=hd_sb, in_=HD_d[:, b])
            xblk = bpool.tile([128, C, 2, 128], BF)
            nc.sync.dma_start(out=xblk, in_=X_d[:, b * C:(b + 1) * C])

            qps = qpp.tile([128, 256], F32)
            nc.tensor.matmul(qps, hd_sb[:, 0, :], wq_sb[:, 0, :],
                             start=True, stop=False)
            nc.tensor.matmul(qps, hd_sb[:, 1, :], wq_sb[:, 1, :],
                             start=False, stop=not has_bias)
            if has_bias:
                nc.tensor.matmul(qps, ones_sb, bq_sb, start=False, stop=True)
            q_sb = bpool.tile([128, 256], F32)
            nc.scalar.activation(q_sb, qps, Copy)

            ups = upp.tile([128, 260], F32)
            for c in range(C):
                kv = kpp.tile([128, 512], F32)
                nc.tensor.matmul(kv, xblk[:, c, 0, :], wkv_sb[:, 0, :],
                                 start=True, stop=False)
                nc.tensor.matmul(kv, xblk[:, c, 1, :], wkv_sb[:, 1, :],
                                 start=False, stop=not has_bias)
                if has_bias:
                    nc.tensor.matmul(kv, ones_sb, bkv_sb, start=False, stop=True)
                a2_sb = kpool.tile([128, 128], F32)
                nc.vector.tensor_scalar(a2_sb, iota_sb, dloc_sb[:, b, c:c + 1],
                                        None, iseq)
                a1ps = app.tile([128, 128], F32)
                nc.tensor.matmul(a1ps, a2_sb, ident_sb, start=True, stop=True)
                a1_sb = kpool.tile([128, 128], F32)
                nc.scalar.activation(a1_sb, a1ps, Copy)
                qg = gpp.tile([128, 256], F32)
                nc.tensor.matmul(qg, a1_sb, q_sb, start=True, stop=True)
                qg_sb = kpool.tile([128, 256], F32)
                nc.scalar.activation(qg_sb, qg, Copy)
                prod = kpool.tile([128, 256], F32)
                nc.vector.tensor_tensor(prod, kv[:, 0:256], qg_sb, mult)
                sc = kpool.tile([128, 4], F32)
                nc.vector.tensor_reduce(sc, prod.rearrange("p (h d) -> p h d", h=4),
                                        mybir.AxisListType.X, addop)
                es = kpool.tile([128, 4], F32)
                nc.scalar.activation(es, sc, Exp, scale=0.125)
                pcat = kpool.tile([128, 260], F32)
                nc.vector.tensor_scalar(pcat[:, 256:260], es, 0.0, None, addop)
                for h in range(H):
                    nc.vector.tensor_scalar(
                        pcat[:, h * 64:(h + 1) * 64],
                        kv[:, 256 + h * 64:256 + (h + 1) * 64],
                        es[:, h:h + 1], None, mult)
                nc.tensor.matmul(ups, a2_sb, pcat,
                                 start=(c == 0), stop=(c == C - 1))

            s_sb = bpool.tile([128, 4], F32)
            nc.vector.tensor_scalar(s_sb, ups[:, 256:260], 1e-30, None, maxop)
            r_sb = bpool.tile([128, 4], F32)
            nc.vector.reciprocal(r_sb, s_sb)
            o_sb = bpool.tile([128, 256], BF)
            for h in range(H):
                nc.vector.tensor_scalar(o_sb[:, h * 64:(h + 1) * 64],
                                        ups[:, h * 64:(h + 1) * 64],
                                        r_sb[:, h:h + 1], None, mult)
            nc.sync.dma_start(out=out_d[b], in_=o_sb)
    return nc


def _run_overlapped(arrs, C, has_bias):
    """Inline of run_bass_kernel_spmd's axon path, with H2D transfers started
    before kernel build + compile so they overlap, and the donated output
    zero-buffers created directly on device."""
    import jax
    import jax.numpy as jnp
    from jax.sharding import Mesh, PartitionSpec, NamedSharding
    from jax.experimental.shard_map import shard_map
    from concourse import bass2jax, mybir

    try:
        jax.config.update("jax_compilation_cache_dir", "/tmp/jax_comp_cache")
        jax.config.update("jax_persistent_cache_min_entry_size_bytes", -1)
        jax.config.update("jax_persistent_cache_min_compile_time_secs", 0.0)
    except Exception:
        pass

    t0 = time.time()
    devices = jax.devices()[:NDEV]
    mesh = Mesh(np.asarray(devices), ("core",))
    sh = NamedSharding(mesh, PartitionSpec("core"))
    # biggest first so the tunnel starts on the critical bytes immediately
    put_order = ["X", "HD", "DLOC", "IOTA", "IDENT", "WKV", "WQ", "BKV", "BQ"]
    darrs = {name: jax.device_put(arrs[name], sh) for name in put_order}
    _tlog("device_put dispatch", t0)

    t0 = time.time()
    nc = _build(C, has_bias)
    nc.finalize()
    _tlog("build", t0)

    t0 = time.time()
    bass2jax.install_neuronx_cc_hook()
    assert nc.dbg_addr is None
    partition_name = nc.partition_id_tensor.name if nc.partition_id_tensor else None
    in_names, out_names, out_avals = [], [], []
    for alloc in nc.m.functions[0].allocations:
        if not isinstance(alloc, mybir.MemoryLocationSet):
            continue
        name = alloc.memorylocations[0].name
        if alloc.kind == "ExternalInput":
            if name != partition_name:
                in_names.append(name)
        elif alloc.kind == "ExternalOutput":
            out_names.append(name)
            shape = tuple(alloc.tensor_shape)
            dtype = mybir.dt.np(alloc.dtype)
            out_avals.append(jax.core.ShapedArray(shape, dtype))
    n_params = len(in_names)
    n_outs = len(out_avals)
    all_names = in_names + out_names
    if partition_name is not None:
        all_names.append(partition_name)
    donate = tuple(range(n_params, n_params + n_outs))

    def _body(*args):
        operands = list(args)
        if partition_name is not None:
            operands.append(bass2jax.partition_id_tensor())
        outs = bass2jax._bass_exec_p.bind(
            *operands,
            out_avals=tuple(out_avals),
            in_names=tuple(all_names),
            out_names=tuple(out_names),
            lowering_input_output_aliases=(),
            sim_require_finite=True,
            sim_require_nnan=True,
            nc=nc,
        )
        return tuple(outs)

    in_specs = (PartitionSpec("core"),) * (n_params + n_outs)
    out_specs = (PartitionSpec("core"),) * n_outs
    sharded = jax.jit(
        shard_map(_body, mesh=mesh, in_specs=in_specs, out_specs=out_specs,
                  check_rep=False),
        donate_argnums=donate, keep_unused=True)

    zeros = [
        jax.jit(lambda a=a: jnp.zeros((NDEV * a.shape[0], *a.shape[1:]),
                                      a.dtype), out_shardings=sh)()
        for a in out_avals
    ]
    args = [darrs[n] for n in in_names] + zeros
    compiled = sharded.lower(*args).compile()
    _tlog("jit compile", t0)

    t0 = time.time()
    out_arrs = compiled(*args)
    jax.block_until_ready(out_arrs)
    _tlog("exec(+transfer wait)", t0)
    t0 = time.time()
    outs = {name: np.asarray(out_arrs[i]) for i, name in enumerate(out_names)}
    _tlog("fetch", t0)
    return outs


def _emulate(arrs, C, has_bias):
    nchunks = NBLK * C
    iota = np.arange(128, dtype=np.float32)
    out_all = np.zeros((NDEV * NBLK, 128, 256), np.float32)
    for d in range(NDEV):
        X = arrs["X"][d * 128:(d + 1) * 128].astype(np.float32)
        DL = arrs["DLOC"][d * 128:(d + 1) * 128]
        HDt = arrs["HD"][d * 128:(d + 1) * 128].astype(np.float32)
        WKV = arrs["WKV"][:128].astype(np.float32)
        WQ = arrs["WQ"][:128].astype(np.float32)
        BKV = arrs["BKV"][0:1].astype(np.float32)
        BQ = arrs["BQ"][0:1].astype(np.float32)
        for b in range(NBLK):
            hd = HDt[:, b]  # [128e, 2, 128d]
            Q = hd[:, 0, :].T @ WQ[:, 0, :] + hd[:, 1, :].T @ WQ[:, 1, :]
            if has_bias:
                Q = Q + BQ
            U = np.zeros((128, 260), np.float32)
            for c in range(C):
                x = X[:, b * C + c]  # [128e, 2, 128slot]
                kv = np.einsum('es,ef->sf', x[:, 0, :], WKV[:, 0, :]) + \
                     np.einsum('es,ef->sf', x[:, 1, :], WKV[:, 1, :])
                if has_bias:
                    kv = kv + BKV
                dloc = DL[:, b, c]  # [128slot]
                a2 = (iota[None, :] == dloc[:, None]).astype(np.float32)
                qg = a2 @ Q
                sc = (kv[:, :256] * qg).reshape(128, 4, 64).sum(-1)
                p = np.exp(sc * 0.125).astype(np.float32)
                pv = (kv[:, 256:].reshape(128, 4, 64) * p[:, :, None]).reshape(128, 256)
                U += a2.T @ np.concatenate([pv, p], axis=1)
            r = 1.0 / np.maximum(U[:, 256:260], 1e-30)
            out_all[d * NBLK + b] = (
                U[:, :256].reshape(128, 4, 64) * r[:, :, None]).reshape(128, 256)
    return {"out": out_all.astype(BF16)}


def kernel(**inputs):
    global LAST_EXEC_NS
    h_src = np.asarray(inputs["h_src"], np.float32)
    h_dst = np.asarray(inputs["h_dst"], np.float32)
    src_idx = np.asarray(inputs["src_idx"]).astype(np.int64)
    dst_idx = np.asarray(inputs["dst_idx"]).astype(np.int64)
    Wq = np.asarray(inputs["Wq"], np.float32)
    bq = np.asarray(inputs["bq"], np.float32)
    Wk = np.asarray(inputs["Wk"], np.float32)
    bk = np.asarray(inputs["bk"], np.float32)
    Wv = np.asarray(inputs["Wv"], np.float32)
    bv = np.asarray(inputs["bv"], np.float32)

    t0 = time.time()
    arrs, C, has_bias = _prep_host(h_src, h_dst, src_idx, dst_idx,
                                   Wq, bq, Wk, bk, Wv, bv)
    _tlog("prep_host", t0)

    if os.environ.get("KERNEL_EMULATE"):
        outs = _emulate(arrs, C, has_bias)
    else:
        outs = _run_overlapped(arrs, C, has_bias)
        LAST_EXEC_NS = None

    out = np.asarray(outs["out"]).reshape(NDEV, NBLK * 128, 256)
    parts = [out[d, :DST_PER_DEV] for d in range(NDEV)]
    return np.ascontiguousarray(
        np.concatenate(parts, axis=0).astype(np.float32))
